# revision 16
# baseline (speedup 1.0000x reference)
"""BinaryTreeLSTM Trainium2 kernel — data-parallel over the batch (tree) axis.

Layout strategy: keep every activation in [feature, row] layout on-chip.
The TensorE matmul produces PSUM [out_feat, row], which is exactly the
moving-operand layout ([K, N], K on partitions) needed by the next tree
level — so no transposes are ever needed on-device.  The host transposes
embs once (numpy) and packs/casts the gate weights.

Per core (64 trees): leaf GEMM (K=300 -> 512 feats = [c|o]), then 8
levelized compose GEMMs (K=512 = [lh|rh] feats -> 1024 = 4 gates x 256).
lh/rh are stride-2 (even/odd node) column slices of the previous level's
h buffer.  Trees are processed in chunks of 8 so leaf/L1/L2 intermediates
fit in SBUF; L3 outputs are parked, then levels 4..8 run across all 64
trees at once to keep matmul free dims large.

Throughput structure:
 - Wide levels process PAIRS of 512-row sub-chunks with [128,1024] PSUM
   tiles (2 banks) so each LDWEIGHTS feeds two matmuls — the 2-deep PE
   weight buffer otherwise serializes LDW behind the prior matmul drain.
 - Emission is software-pipelined across chunks (leaf(ch) | L2(ch-1) |
   L1(ch) | L3(ch-2)) so the PE never waits on the serial ACT->DVE chain
   between adjacent tree levels; tanh(c)->h is deferred and batched per
   level.
 - A burst of dummy matmuls at kernel start warms the PE HAM clock gate
   during the initial DMA wait.

dtypes: matmul operands bf16 (fp32 PSUM accumulate); the c cell state and
gate activations stay fp32 end-to-end.
"""

import sys

if "/opt/trn_rl_repo" not in sys.path:
    sys.path.insert(0, "/opt/trn_rl_repo")

from contextlib import ExitStack

import ml_dtypes
import numpy as np

N_CORES = 8
B, L, IN, M = 512, 256, 300, 256
BC = B // N_CORES            # trees per core
LEAF_ROWS = BC * L           # 16384 leaf rows per core
T_CHUNK = 8                  # trees per chunk
N_CHUNKS = BC // T_CHUNK
CHUNK_LEAF = T_CHUNK * L     # 2048 leaf rows per chunk

_CACHE = {}
LAST_RESULTS = None


def _build():
    import concourse.bass as bass  # noqa: F401
    import concourse.tile as tile
    from concourse import bacc, mybir

    F32 = mybir.dt.float32
    BF16 = mybir.dt.bfloat16
    SIG = mybir.ActivationFunctionType.Sigmoid
    TANH = mybir.ActivationFunctionType.Tanh
    ADD = mybir.AluOpType.add

    nc = bacc.Bacc("TRN2", target_bir_lowering=False, debug=False,
                   num_devices=N_CORES)
    emb_d = nc.dram_tensor("embs_t", [IN, LEAF_ROWS], BF16,
                           kind="ExternalInput").ap()
    wl_d = nc.dram_tensor("w_leaf", [IN, 2 * M], BF16,
                          kind="ExternalInput").ap()
    wc_d = nc.dram_tensor("w_comp", [2 * M, 4 * M], BF16,
                          kind="ExternalInput").ap()
    b_d = nc.dram_tensor("biases", [128, 12], F32, kind="ExternalInput").ap()
    outc_d = nc.dram_tensor("out_c", [M, BC], F32, kind="ExternalOutput").ap()
    outh_d = nc.dram_tensor("out_h", [M, BC], F32, kind="ExternalOutput").ap()

    with tile.TileContext(nc) as tc, ExitStack() as ctx:
        wpool = ctx.enter_context(tc.tile_pool(name="w", bufs=1))
        xpool = ctx.enter_context(tc.tile_pool(name="x", bufs=4))
        lvl = ctx.enter_context(tc.tile_pool(name="lvl", bufs=1))
        gp = ctx.enter_context(tc.tile_pool(name="g", bufs=2))
        pp = ctx.enter_context(tc.tile_pool(name="ps", bufs=1, space="PSUM"))

        # --- weights / biases (resident); issued on the ACT HWDGE queue so
        # the SP queue leads with the first leaf's embs tiles ---
        wl = [wpool.tile([128, 2 * M], BF16, name=f"wl{k}", tag=f"wl{k}")
              for k in range(3)]
        nc.scalar.dma_start(wl[0][:], wl_d[0:128, :])
        nc.scalar.dma_start(wl[1][:], wl_d[128:256, :])
        nc.scalar.dma_start(wl[2][:44, :], wl_d[256:300, :])
        wc = [wpool.tile([128, 4 * M], BF16, name=f"wc{k}", tag=f"wc{k}")
              for k in range(4)]
        bias = wpool.tile([128, 12], F32, name="bias", tag="bias")
        nc.scalar.dma_start(bias[:], b_d[:, :])

        # PSUM: four [128, 1024] tiles = 2 banks each = all 8 banks
        def pa_tiles():
            return [pp.tile([128, 1024], F32, name=f"pa{i}", tag=f"pa{i}")
                    for i in range(4)]

        # --- HAM warm-up: dummy matmuls during the startup DMA wait ---
        dummy = wpool.tile([128, 512], BF16, name="dummy", tag="dummy")
        nc.gpsimd.memset(dummy[:], 0.0)
        warm = pa_tiles()
        for i in range(24):
            nc.tensor.matmul(warm[i % 4][:, 0:512], dummy[:, 0:128],
                             dummy[:], start=True, stop=True)

        # --- parked L3 state (all trees) ---
        pk_w = BC * 32
        pk_h = [lvl.tile([128, pk_w], BF16, name=f"pkh{p}", tag=f"pkh{p}")
                for p in range(2)]
        pk_c = [lvl.tile([128, pk_w], F32, name=f"pkc{p}", tag=f"pkc{p}")
                for p in range(2)]

        # gate m-tile order: the gates needed first by the DVE c-chain
        # (i, u, lf, rf of part-half 0) drain first.
        MT_ORDER = (0, 6, 2, 4, 1, 7, 3, 5)

        def _gates_and_chain(ps, ce, houts, couts, n, defer_h):
            """ps[mt]: psum APs [128, n]; ce[p]: even/odd c view
            [128, 2, n]; gates + c-chain for one <=512-row sub-chunk."""
            g = [None] * 8
            for mt in MT_ORDER:
                gt = gp.tile([128, 512], F32, name=f"g{mt}", tag=f"g{mt}")
                fn = TANH if mt >= 6 else SIG
                nc.scalar.activation(gt[:, :n], ps[mt], fn,
                                     bias=bias[:, 4 + mt:5 + mt])
                g[mt] = gt
            for p in range(2):
                t1 = gp.tile([128, 512], F32, name=f"t1_{p}", tag=f"t1_{p}",
                             bufs=1)
                nc.vector.tensor_mul(t1[:, :n], g[0 + p][:, :n],
                                     g[6 + p][:, :n])
                t2 = gp.tile([128, 512], F32, name=f"t2_{p}", tag=f"t2_{p}",
                             bufs=1)
                nc.vector.tensor_mul(t2[:, :n], g[2 + p][:, :n],
                                     ce[p][:, 0, :])
                t3 = gp.tile([128, 512], F32, name=f"t3_{p}", tag=f"t3_{p}",
                             bufs=1)
                nc.vector.tensor_mul(t3[:, :n], g[4 + p][:, :n],
                                     ce[p][:, 1, :])
                s12 = gp.tile([128, 512], F32, name=f"s12_{p}",
                              tag=f"s12_{p}", bufs=1)
                nc.vector.tensor_add(s12[:, :n], t1[:, :n], t2[:, :n])
                nc.vector.tensor_add(couts[p], s12[:, :n], t3[:, :n])
                if not defer_h:
                    nc.scalar.activation(houts[p], couts[p], TANH)

        def compose(hp, cp, houts, couts, n, defer_h=False):
            """One narrow compose sub-chunk (n <= 512 output rows).
            psum m-tile mt lives in pa[mt//2], half mt%2 (pair partners sit
            4 apart in MT_ORDER, giving the gate time to drain)."""
            he = [hp[p].rearrange("q (n two) -> q two n", two=2)
                  for p in range(2)]
            ce = [cp[p].rearrange("q (n two) -> q two n", two=2)
                  for p in range(2)]
            rhs = [he[0][:, 0, :], he[1][:, 0, :],
                   he[0][:, 1, :], he[1][:, 1, :]]
            pa = pa_tiles()
            ps = [pa[mt // 2][:, (mt % 2) * 512:(mt % 2) * 512 + n]
                  for mt in range(8)]
            for mt in MT_ORDER:
                for k in range(4):
                    nc.tensor.matmul(ps[mt],
                                     wc[k][:, mt * 128:(mt + 1) * 128],
                                     rhs[k], start=(k == 0), stop=(k == 3))
            _gates_and_chain(ps, ce, houts, couts, n, defer_h)

        st = {}  # (level, ch) -> (h_tiles, c_tiles)

        def emit_leaf(ch):
            h_lf = [lvl.tile([128, CHUNK_LEAF], BF16, name=f"hlf{p}",
                             tag=f"hlf{p}") for p in range(2)]
            c_lf = [lvl.tile([128, CHUNK_LEAF], F32, name=f"clf{p}",
                             tag=f"clf{p}") for p in range(2)]
            for sp in range(2):          # two pairs of 512-row sub-chunks
                col0 = ch * CHUNK_LEAF + sp * 1024
                xk = [[xpool.tile([128, 512], BF16, name=f"xk{k}",
                                  tag=f"xk{k}") for k in range(3)]
                      for _ in range(2)]
                for j in range(2):
                    c0 = col0 + j * 512
                    nc.sync.dma_start(xk[j][0][:], emb_d[0:128, c0:c0 + 512])
                    nc.sync.dma_start(xk[j][1][:],
                                      emb_d[128:256, c0:c0 + 512])
                    nc.sync.dma_start(xk[j][2][:44, :],
                                      emb_d[256:300, c0:c0 + 512])
                pa = pa_tiles()
                for mt in range(4):
                    for k in range(3):
                        kp = 128 if k < 2 else 44
                        for j in range(2):
                            nc.tensor.matmul(
                                pa[mt][:, j * 512:(j + 1) * 512],
                                wl[k][:kp, mt * 128:(mt + 1) * 128],
                                xk[j][k][:kp, :],
                                start=(k == 0), stop=(k == 2))
                for j in range(2):
                    dst = slice(sp * 1024 + j * 512,
                                sp * 1024 + j * 512 + 512)
                    for p in range(2):
                        tcell = gp.tile([128, 512], F32, name=f"lf_tc{p}",
                                        tag=f"lf_tc{p}")
                        nc.scalar.activation(
                            tcell[:], pa[p][:, j * 512:(j + 1) * 512], TANH,
                            bias=bias[:, p:p + 1])
                        to = gp.tile([128, 512], F32, name=f"lf_to{p}",
                                     tag=f"lf_to{p}")
                        nc.scalar.activation(
                            to[:], pa[2 + p][:, j * 512:(j + 1) * 512], SIG,
                            bias=bias[:, 2 + p:3 + p])
                        nc.vector.tensor_scalar(
                            c_lf[p][:, dst],
                            pa[p][:, j * 512:(j + 1) * 512],
                            bias[:, p:p + 1], None, ADD)
                        nc.vector.tensor_mul(h_lf[p][:, dst], to[:],
                                             tcell[:])
            st[(0, ch)] = (h_lf, c_lf)

        def emit_wide(prev_h, prev_c, nh, ncr, off, pairs):
            """Wide compose: pair-calls of 1024 output rows.  Two half-passes
            of 4 m-tiles; within a pass each LDWEIGHTS feeds the two
            sub-chunks' matmuls into one [128,1024] psum tile (4 tiles =
            all 8 banks).  Half-pass `p` produces exactly the gates the
            part-half-p DVE c-chain consumes, so the chain overlaps the
            other half-pass's matmuls."""
            for q in range(pairs):
                hp = [prev_h[p][:, q * 2048:(q + 1) * 2048]
                      for p in range(2)]
                cp = [prev_c[p][:, q * 2048:(q + 1) * 2048]
                      for p in range(2)]
                he = [hp[p].rearrange("q2 (n two) -> q2 two n", two=2)
                      for p in range(2)]
                ce = [cp[p].rearrange("q2 (n two) -> q2 two n", two=2)
                      for p in range(2)]
                rhs = [[he[0][:, 0, j * 512:(j + 1) * 512] for j in range(2)],
                       [he[1][:, 0, j * 512:(j + 1) * 512] for j in range(2)],
                       [he[0][:, 1, j * 512:(j + 1) * 512] for j in range(2)],
                       [he[1][:, 1, j * 512:(j + 1) * 512] for j in range(2)]]
                for p in range(2):
                    mts = MT_ORDER[4 * p:4 * p + 4]
                    pa = pa_tiles()
                    for idx, mt in enumerate(mts):
                        pt = pa[idx]
                        for k in range(4):
                            for j in range(2):
                                nc.tensor.matmul(
                                    pt[:, j * 512:(j + 1) * 512],
                                    wc[k][:, mt * 128:(mt + 1) * 128],
                                    rhs[k][j],
                                    start=(k == 0), stop=(k == 3))
                    g = {}
                    for idx, mt in enumerate(mts):
                        for j in range(2):
                            gt = gp.tile([128, 512], F32, name=f"g{mt}",
                                         tag=f"g{mt}")
                            fn = TANH if mt >= 6 else SIG
                            nc.scalar.activation(
                                gt[:], pa[idx][:, j * 512:(j + 1) * 512],
                                fn, bias=bias[:, 4 + mt:5 + mt])
                            g[(mt, j)] = gt
                    for j in range(2):
                        cej = ce[p][:, :, j * 512:(j + 1) * 512]
                        cout = ncr[p][:, off + q * 1024 + j * 512:
                                      off + q * 1024 + j * 512 + 512]
                        t1 = gp.tile([128, 512], F32, name=f"t1_{p}",
                                     tag=f"t1_{p}", bufs=1)
                        nc.vector.tensor_mul(t1[:], g[(0 + p, j)][:],
                                             g[(6 + p, j)][:])
                        t2 = gp.tile([128, 512], F32, name=f"t2_{p}",
                                     tag=f"t2_{p}", bufs=1)
                        nc.vector.tensor_mul(t2[:], g[(2 + p, j)][:],
                                             cej[:, 0, :])
                        t3 = gp.tile([128, 512], F32, name=f"t3_{p}",
                                     tag=f"t3_{p}", bufs=1)
                        nc.vector.tensor_mul(t3[:], g[(4 + p, j)][:],
                                             cej[:, 1, :])
                        s12 = gp.tile([128, 512], F32, name=f"s12_{p}",
                                      tag=f"s12_{p}", bufs=1)
                        nc.vector.tensor_add(s12[:], t1[:], t2[:])
                        nc.vector.tensor_add(cout, s12[:], t3[:])

        def emit_level(li, ch):
            """Compose level li (1..3) of chunk ch."""
            prev_h, prev_c = st.pop((li - 1, ch))
            rows = CHUNK_LEAF >> li
            if li < 3:
                nh = [lvl.tile([128, rows], BF16, name=f"h{li}_{p}",
                               tag=f"h{li}_{p}", bufs=2) for p in range(2)]
                ncr = [lvl.tile([128, rows], F32, name=f"c{li}_{p}",
                                tag=f"c{li}_{p}", bufs=2) for p in range(2)]
                off = 0
            else:
                nh, ncr, off = pk_h, pk_c, ch * (CHUNK_LEAF // 8)
            if rows >= 1024:
                emit_wide(prev_h, prev_c, nh, ncr, off, rows // 1024)
            else:
                for j in range(max(1, rows // 512)):
                    n = min(512, rows - j * 512)
                    hp = [prev_h[p][:, j * 1024:j * 1024 + 2 * n]
                          for p in range(2)]
                    cp = [prev_c[p][:, j * 1024:j * 1024 + 2 * n]
                          for p in range(2)]
                    houts = [nh[p][:, off + j * 512:off + j * 512 + n]
                             for p in range(2)]
                    couts = [ncr[p][:, off + j * 512:off + j * 512 + n]
                             for p in range(2)]
                    compose(hp, cp, houts, couts, n, defer_h=True)
            for p in range(2):
                nc.scalar.activation(nh[p][:, off:off + rows],
                                     ncr[p][:, off:off + rows], TANH)
            if li < 3:
                st[(li, ch)] = (nh, ncr)

        # software-pipelined chunk schedule
        for ch in range(N_CHUNKS):
            emit_leaf(ch)
            if ch == 0:
                for k in range(4):
                    nc.scalar.dma_start(wc[k][:],
                                        wc_d[k * 128:(k + 1) * 128, :])
            if ch >= 1:
                emit_level(2, ch - 1)
            emit_level(1, ch)
            if ch >= 2:
                emit_level(3, ch - 2)
        emit_level(2, N_CHUNKS - 1)
        emit_level(3, N_CHUNKS - 2)

        # ---- tail levels 4..8 across all trees ----
        def tail_tiles(li, rows):
            nh = [lvl.tile([128, rows], BF16, name=f"h{li}_{p}",
                           tag=f"h{li}_{p}") for p in range(2)]
            ncr = [lvl.tile([128, rows], F32, name=f"c{li}_{p}",
                            tag=f"c{li}_{p}") for p in range(2)]
            return nh, ncr

        def tail_sub(prev, cur, j, n):
            prev_h, prev_c = prev
            nh, ncr = cur
            hp = [prev_h[p][:, j * 2 * n:j * 2 * n + 2 * n]
                  for p in range(2)]
            cp = [prev_c[p][:, j * 2 * n:j * 2 * n + 2 * n]
                  for p in range(2)]
            houts = [nh[p][:, j * n:j * n + n] for p in range(2)]
            couts = [ncr[p][:, j * n:j * n + n] for p in range(2)]
            compose(hp, cp, houts, couts, n, defer_h=True)
            for p in range(2):
                nc.scalar.activation(houts[p], couts[p], TANH)

        l4 = tail_tiles(4, BC * 16)
        l5 = tail_tiles(5, BC * 8)
        l6 = tail_tiles(6, BC * 4)
        l7 = tail_tiles(7, BC * 2)
        tail_sub((pk_h, pk_c), l4, 0, 512)
        emit_level(3, N_CHUNKS - 1)
        tail_sub((pk_h, pk_c), l4, 1, 512)
        tail_sub(l4, l5, 0, 256)
        tail_sub(l4, l5, 1, 256)
        tail_sub(l5, l6, 0, 128)
        tail_sub(l5, l6, 1, 128)
        tail_sub(l6, l7, 0, 64)
        tail_sub(l6, l7, 1, 64)
        stage_c = [lvl.tile([128, BC], F32, name=f"stc{p}", tag=f"stc{p}")
                   for p in range(2)]
        stage_h = [lvl.tile([128, BC], F32, name=f"sth{p}", tag=f"sth{p}")
                   for p in range(2)]
        for j in range(2):
            compose([l7[0][p][:, j * 64:j * 64 + 64] for p in range(2)],
                    [l7[1][p][:, j * 64:j * 64 + 64] for p in range(2)],
                    [stage_h[p][:, j * 32:(j + 1) * 32] for p in range(2)],
                    [stage_c[p][:, j * 32:(j + 1) * 32] for p in range(2)],
                    32)
        for p in range(2):
            nc.sync.dma_start(outc_d[p * 128:(p + 1) * 128, :], stage_c[p][:])
            nc.sync.dma_start(outh_d[p * 128:(p + 1) * 128, :], stage_h[p][:])

    nc.compile()
    return nc


def _prep_inputs(embs, cx_w, cx_b, ox_w, ox_b, lh_w, lh_b, rh_w, rh_b):
    bf16 = ml_dtypes.bfloat16
    w_leaf = np.ascontiguousarray(
        np.concatenate([cx_w, ox_w], axis=0).T).astype(bf16)      # [300, 512]
    w_comp = np.ascontiguousarray(np.concatenate(
        [lh_w.reshape(4 * M, M).T, rh_w.reshape(4 * M, M).T],
        axis=0)).astype(bf16)                                     # [512, 1024]
    b_all = np.concatenate(
        [cx_b, ox_b, (lh_b + rh_b).reshape(-1)]).astype(np.float32)
    biases = np.ascontiguousarray(b_all.reshape(12, 128).T)       # [128, 12]
    embs_sh = np.asarray(embs, np.float32).reshape(N_CORES, LEAF_ROWS, IN)
    in_maps = []
    for c in range(N_CORES):
        xt = embs_sh[c].T.astype(bf16)                            # [300, 16384]
        in_maps.append({"embs_t": np.ascontiguousarray(xt),
                        "w_leaf": w_leaf, "w_comp": w_comp,
                        "biases": biases})
    return in_maps


def kernel(embs, cx_w, cx_b, ox_w, ox_b, lh_w, lh_b, rh_w, rh_b):
    global LAST_RESULTS
    from concourse.bass_utils import run_bass_kernel_spmd

    if "nc" not in _CACHE:
        _CACHE["nc"] = _build()
    nc = _CACHE["nc"]
    in_maps = _prep_inputs(embs, cx_w, cx_b, ox_w, ox_b,
                           lh_w, lh_b, rh_w, rh_b)
    res = run_bass_kernel_spmd(nc, in_maps, core_ids=list(range(N_CORES)))
    LAST_RESULTS = res
    c_out = np.empty((B, M), np.float32)
    h_out = np.empty((B, M), np.float32)
    for c in range(N_CORES):
        c_out[c * BC:(c + 1) * BC] = res.results[c]["out_c"].T
        h_out[c * BC:(c + 1) * BC] = res.results[c]["out_h"].T
    return c_out, h_out


# revision 17
# speedup vs baseline: 1.0686x; 1.0686x over previous
"""BinaryTreeLSTM Trainium2 kernel — data-parallel over the batch (tree) axis.

Layout strategy: keep every activation in [feature, row] layout on-chip.
The TensorE matmul produces PSUM [out_feat, row], which is exactly the
moving-operand layout ([K, N], K on partitions) needed by the next tree
level — so no transposes are ever needed on-device.  The host transposes
embs once (numpy) and packs/casts the gate weights.

Per core (64 trees): leaf GEMM (K=300 -> 512 feats = [c|o]), then 8
levelized compose GEMMs (K=512 = [lh|rh] feats -> 1024 = 4 gates x 256).
lh/rh are stride-2 (even/odd node) column slices of the previous level's
h buffer.  Trees are processed in chunks of 8 so leaf/L1/L2 intermediates
fit in SBUF; L3 outputs are parked, then levels 4..8 run across all 64
trees at once to keep matmul free dims large.

Throughput structure:
 - Wide levels process PAIRS of 512-row sub-chunks with [128,1024] PSUM
   tiles (2 banks) so each LDWEIGHTS feeds two matmuls — the 2-deep PE
   weight buffer otherwise serializes LDW behind the prior matmul drain.
 - Emission is software-pipelined across chunks (leaf(ch) | L2(ch-1) |
   L1(ch) | L3(ch-2)) so the PE never waits on the serial ACT->DVE chain
   between adjacent tree levels; tanh(c)->h is deferred and batched per
   level.
 - A burst of dummy matmuls at kernel start warms the PE HAM clock gate
   during the initial DMA wait.

dtypes: matmul operands bf16 (fp32 PSUM accumulate); the c cell state and
gate activations stay fp32 end-to-end.
"""

import sys

if "/opt/trn_rl_repo" not in sys.path:
    sys.path.insert(0, "/opt/trn_rl_repo")

from contextlib import ExitStack

import ml_dtypes
import numpy as np

N_CORES = 8
B, L, IN, M = 512, 256, 300, 256
BC = B // N_CORES            # trees per core
LEAF_ROWS = BC * L           # 16384 leaf rows per core
T_CHUNK = 8                  # trees per chunk
N_CHUNKS = BC // T_CHUNK
CHUNK_LEAF = T_CHUNK * L     # 2048 leaf rows per chunk

_CACHE = {}
LAST_RESULTS = None


def _build():
    import concourse.bass as bass  # noqa: F401
    import concourse.tile as tile
    from concourse import bacc, mybir

    F32 = mybir.dt.float32
    BF16 = mybir.dt.bfloat16
    SIG = mybir.ActivationFunctionType.Sigmoid
    TANH = mybir.ActivationFunctionType.Tanh
    ADD = mybir.AluOpType.add

    nc = bacc.Bacc("TRN2", target_bir_lowering=False, debug=False,
                   num_devices=N_CORES)
    emb_d = nc.dram_tensor("embs_t", [IN, LEAF_ROWS], BF16,
                           kind="ExternalInput").ap()
    wl_d = nc.dram_tensor("w_leaf", [IN, 2 * M], BF16,
                          kind="ExternalInput").ap()
    wc_d = nc.dram_tensor("w_comp", [2 * M, 4 * M], BF16,
                          kind="ExternalInput").ap()
    b_d = nc.dram_tensor("biases", [128, 12], F32, kind="ExternalInput").ap()
    outc_d = nc.dram_tensor("out_c", [M, BC], F32, kind="ExternalOutput").ap()
    outh_d = nc.dram_tensor("out_h", [M, BC], F32, kind="ExternalOutput").ap()

    with tile.TileContext(nc) as tc, ExitStack() as ctx:
        wpool = ctx.enter_context(tc.tile_pool(name="w", bufs=1))
        xpool = ctx.enter_context(tc.tile_pool(name="x", bufs=4))
        lvl = ctx.enter_context(tc.tile_pool(name="lvl", bufs=1))
        gp = ctx.enter_context(tc.tile_pool(name="g", bufs=2))
        pp = ctx.enter_context(tc.tile_pool(name="ps", bufs=1, space="PSUM"))

        # --- weights / biases (resident); issued on the ACT HWDGE queue so
        # the SP queue leads with the first leaf's embs tiles ---
        wl = [wpool.tile([128, 2 * M], BF16, name=f"wl{k}", tag=f"wl{k}")
              for k in range(3)]
        nc.scalar.dma_start(wl[0][:], wl_d[0:128, :])
        nc.scalar.dma_start(wl[1][:], wl_d[128:256, :])
        nc.scalar.dma_start(wl[2][:44, :], wl_d[256:300, :])
        wc = [wpool.tile([128, 4 * M], BF16, name=f"wc{k}", tag=f"wc{k}")
              for k in range(4)]
        bias = wpool.tile([128, 12], F32, name="bias", tag="bias")
        nc.scalar.dma_start(bias[:], b_d[:, :])

        # PSUM: eight [128, 512] tiles = one bank each
        def ps_tile(i, n=512):
            t = pp.tile([128, 512], F32, name=f"ps{i}", tag=f"ps{i}")
            return t[:, :n] if n < 512 else t

        # --- HAM warm-up: dummy matmuls during the startup DMA wait ---
        dummy = wpool.tile([128, 512], BF16, name="dummy", tag="dummy")
        nc.gpsimd.memset(dummy[:], 0.0)
        for i in range(24):
            nc.tensor.matmul(ps_tile(i % 8), dummy[:, 0:128],
                             dummy[:], start=True, stop=True)

        # --- parked L3 state (all trees) ---
        pk_w = BC * 32
        pk_h = [lvl.tile([128, pk_w], BF16, name=f"pkh{p}", tag=f"pkh{p}")
                for p in range(2)]
        pk_c = [lvl.tile([128, pk_w], F32, name=f"pkc{p}", tag=f"pkc{p}")
                for p in range(2)]

        # gate m-tile order: the gates needed first by the DVE c-chain
        # (i, u, lf, rf of part-half 0) drain first.
        MT_ORDER = (0, 6, 2, 4, 1, 7, 3, 5)

        def _gates_and_chain(ps, ce, houts, couts, n, defer_h):
            """ps[mt]: psum APs [128, n]; ce[p]: even/odd c view
            [128, 2, n]; gates + c-chain for one <=512-row sub-chunk."""
            g = [None] * 8
            for mt in MT_ORDER:
                gt = gp.tile([128, 512], F32, name=f"g{mt}", tag=f"g{mt}")
                fn = TANH if mt >= 6 else SIG
                nc.scalar.activation(gt[:, :n], ps[mt], fn,
                                     bias=bias[:, 4 + mt:5 + mt])
                g[mt] = gt
            for p in range(2):
                t1 = gp.tile([128, 512], F32, name=f"t1_{p}", tag=f"t1_{p}",
                             bufs=1)
                nc.vector.tensor_mul(t1[:, :n], g[0 + p][:, :n],
                                     g[6 + p][:, :n])
                t2 = gp.tile([128, 512], F32, name=f"t2_{p}", tag=f"t2_{p}",
                             bufs=1)
                nc.vector.tensor_mul(t2[:, :n], g[2 + p][:, :n],
                                     ce[p][:, 0, :])
                t3 = gp.tile([128, 512], F32, name=f"t3_{p}", tag=f"t3_{p}",
                             bufs=1)
                nc.vector.tensor_mul(t3[:, :n], g[4 + p][:, :n],
                                     ce[p][:, 1, :])
                s12 = gp.tile([128, 512], F32, name=f"s12_{p}",
                              tag=f"s12_{p}", bufs=1)
                nc.vector.tensor_add(s12[:, :n], t1[:, :n], t2[:, :n])
                nc.vector.tensor_add(couts[p], s12[:, :n], t3[:, :n])
                if not defer_h:
                    nc.scalar.activation(houts[p], couts[p], TANH)

        def compose(hp, cp, houts, couts, n, defer_h=False):
            """One compose sub-chunk (n <= 512 output rows)."""
            he = [hp[p].rearrange("q (n two) -> q two n", two=2)
                  for p in range(2)]
            ce = [cp[p].rearrange("q (n two) -> q two n", two=2)
                  for p in range(2)]
            rhs = [he[0][:, 0, :], he[1][:, 0, :],
                   he[0][:, 1, :], he[1][:, 1, :]]
            ps = [ps_tile(mt, n) for mt in range(8)]
            for mt in MT_ORDER:
                for k in range(4):
                    nc.tensor.matmul(ps[mt],
                                     wc[k][:, mt * 128:(mt + 1) * 128],
                                     rhs[k], start=(k == 0), stop=(k == 3))
            _gates_and_chain(ps, ce, houts, couts, n, defer_h)

        st = {}  # (level, ch) -> (h_tiles, c_tiles)

        def emit_leaf(ch):
            h_lf = [lvl.tile([128, CHUNK_LEAF], BF16, name=f"hlf{p}",
                             tag=f"hlf{p}") for p in range(2)]
            c_lf = [lvl.tile([128, CHUNK_LEAF], F32, name=f"clf{p}",
                             tag=f"clf{p}") for p in range(2)]
            for s in range(CHUNK_LEAF // 512):
                col0 = ch * CHUNK_LEAF + s * 512
                xk = [xpool.tile([128, 512], BF16, name=f"xk{k}",
                                 tag=f"xk{k}") for k in range(3)]
                nc.sync.dma_start(xk[0][:], emb_d[0:128, col0:col0 + 512])
                nc.sync.dma_start(xk[1][:], emb_d[128:256, col0:col0 + 512])
                nc.sync.dma_start(xk[2][:44, :],
                                  emb_d[256:300, col0:col0 + 512])
                pso = 4 * (s % 2)
                ps = [ps_tile(pso + mt) for mt in range(4)]
                for mt in range(4):
                    for k in range(3):
                        kp = 128 if k < 2 else 44
                        nc.tensor.matmul(
                            ps[mt],
                            wl[k][:kp, mt * 128:(mt + 1) * 128],
                            xk[k][:kp, :], start=(k == 0), stop=(k == 2))
                dst = slice(s * 512, (s + 1) * 512)
                for p in range(2):
                    tcell = gp.tile([128, 512], F32, name=f"lf_tc{p}",
                                    tag=f"lf_tc{p}")
                    nc.scalar.activation(tcell[:], ps[p], TANH,
                                         bias=bias[:, p:p + 1])
                    to = gp.tile([128, 512], F32, name=f"lf_to{p}",
                                 tag=f"lf_to{p}")
                    nc.scalar.activation(to[:], ps[2 + p], SIG,
                                         bias=bias[:, 2 + p:3 + p])
                    nc.vector.tensor_scalar(c_lf[p][:, dst], ps[p],
                                            bias[:, p:p + 1], None, ADD)
                    nc.vector.tensor_mul(h_lf[p][:, dst], to[:], tcell[:])
            st[(0, ch)] = (h_lf, c_lf)

        def emit_level(li, ch):
            """Compose level li (1..3) of chunk ch."""
            prev_h, prev_c = st.pop((li - 1, ch))
            rows = CHUNK_LEAF >> li
            if li < 3:
                nh = [lvl.tile([128, rows], BF16, name=f"h{li}_{p}",
                               tag=f"h{li}_{p}", bufs=2) for p in range(2)]
                ncr = [lvl.tile([128, rows], F32, name=f"c{li}_{p}",
                                tag=f"c{li}_{p}", bufs=2) for p in range(2)]
                off = 0
            else:
                nh, ncr, off = pk_h, pk_c, ch * (CHUNK_LEAF // 8)
            for j in range(max(1, rows // 512)):
                n = min(512, rows - j * 512)
                hp = [prev_h[p][:, j * 1024:j * 1024 + 2 * n]
                      for p in range(2)]
                cp = [prev_c[p][:, j * 1024:j * 1024 + 2 * n]
                      for p in range(2)]
                houts = [nh[p][:, off + j * 512:off + j * 512 + n]
                         for p in range(2)]
                couts = [ncr[p][:, off + j * 512:off + j * 512 + n]
                         for p in range(2)]
                compose(hp, cp, houts, couts, n, defer_h=True)
            for p in range(2):
                nc.scalar.activation(nh[p][:, off:off + rows],
                                     ncr[p][:, off:off + rows], TANH)
            if li < 3:
                st[(li, ch)] = (nh, ncr)

        # software-pipelined chunk schedule
        for ch in range(N_CHUNKS):
            emit_leaf(ch)
            if ch == 0:
                for k in range(4):
                    nc.scalar.dma_start(wc[k][:],
                                        wc_d[k * 128:(k + 1) * 128, :])
            if ch >= 1:
                emit_level(2, ch - 1)
            emit_level(1, ch)
            if ch >= 2:
                emit_level(3, ch - 2)
        emit_level(2, N_CHUNKS - 1)
        emit_level(3, N_CHUNKS - 2)

        # ---- tail levels 4..8 across all trees ----
        def tail_tiles(li, rows):
            nh = [lvl.tile([128, rows], BF16, name=f"h{li}_{p}",
                           tag=f"h{li}_{p}") for p in range(2)]
            ncr = [lvl.tile([128, rows], F32, name=f"c{li}_{p}",
                            tag=f"c{li}_{p}") for p in range(2)]
            return nh, ncr

        def tail_sub(prev, cur, j, n):
            prev_h, prev_c = prev
            nh, ncr = cur
            hp = [prev_h[p][:, j * 2 * n:j * 2 * n + 2 * n]
                  for p in range(2)]
            cp = [prev_c[p][:, j * 2 * n:j * 2 * n + 2 * n]
                  for p in range(2)]
            houts = [nh[p][:, j * n:j * n + n] for p in range(2)]
            couts = [ncr[p][:, j * n:j * n + n] for p in range(2)]
            compose(hp, cp, houts, couts, n, defer_h=True)
            for p in range(2):
                nc.scalar.activation(houts[p], couts[p], TANH)

        l4 = tail_tiles(4, BC * 16)
        l5 = tail_tiles(5, BC * 8)
        l6 = tail_tiles(6, BC * 4)
        l7 = tail_tiles(7, BC * 2)
        tail_sub((pk_h, pk_c), l4, 0, 512)
        emit_level(3, N_CHUNKS - 1)
        tail_sub((pk_h, pk_c), l4, 1, 512)
        tail_sub(l4, l5, 0, 256)
        tail_sub(l4, l5, 1, 256)
        tail_sub(l5, l6, 0, 128)
        tail_sub(l5, l6, 1, 128)
        tail_sub(l6, l7, 0, 64)
        tail_sub(l6, l7, 1, 64)
        stage_c = [lvl.tile([128, BC], F32, name=f"stc{p}", tag=f"stc{p}")
                   for p in range(2)]
        stage_h = [lvl.tile([128, BC], F32, name=f"sth{p}", tag=f"sth{p}")
                   for p in range(2)]
        for j in range(2):
            compose([l7[0][p][:, j * 64:j * 64 + 64] for p in range(2)],
                    [l7[1][p][:, j * 64:j * 64 + 64] for p in range(2)],
                    [stage_h[p][:, j * 32:(j + 1) * 32] for p in range(2)],
                    [stage_c[p][:, j * 32:(j + 1) * 32] for p in range(2)],
                    32)
        for p in range(2):
            nc.sync.dma_start(outc_d[p * 128:(p + 1) * 128, :], stage_c[p][:])
            nc.sync.dma_start(outh_d[p * 128:(p + 1) * 128, :], stage_h[p][:])

    nc.compile()
    return nc


def _prep_inputs(embs, cx_w, cx_b, ox_w, ox_b, lh_w, lh_b, rh_w, rh_b):
    bf16 = ml_dtypes.bfloat16
    w_leaf = np.ascontiguousarray(
        np.concatenate([cx_w, ox_w], axis=0).T).astype(bf16)      # [300, 512]
    w_comp = np.ascontiguousarray(np.concatenate(
        [lh_w.reshape(4 * M, M).T, rh_w.reshape(4 * M, M).T],
        axis=0)).astype(bf16)                                     # [512, 1024]
    b_all = np.concatenate(
        [cx_b, ox_b, (lh_b + rh_b).reshape(-1)]).astype(np.float32)
    biases = np.ascontiguousarray(b_all.reshape(12, 128).T)       # [128, 12]
    embs_sh = np.asarray(embs, np.float32).reshape(N_CORES, LEAF_ROWS, IN)
    in_maps = []
    for c in range(N_CORES):
        xt = embs_sh[c].T.astype(bf16)                            # [300, 16384]
        in_maps.append({"embs_t": np.ascontiguousarray(xt),
                        "w_leaf": w_leaf, "w_comp": w_comp,
                        "biases": biases})
    return in_maps


def kernel(embs, cx_w, cx_b, ox_w, ox_b, lh_w, lh_b, rh_w, rh_b):
    global LAST_RESULTS
    from concourse.bass_utils import run_bass_kernel_spmd

    if "nc" not in _CACHE:
        _CACHE["nc"] = _build()
    nc = _CACHE["nc"]
    in_maps = _prep_inputs(embs, cx_w, cx_b, ox_w, ox_b,
                           lh_w, lh_b, rh_w, rh_b)
    res = run_bass_kernel_spmd(nc, in_maps, core_ids=list(range(N_CORES)))
    LAST_RESULTS = res
    c_out = np.empty((B, M), np.float32)
    h_out = np.empty((B, M), np.float32)
    for c in range(N_CORES):
        c_out[c * BC:(c + 1) * BC] = res.results[c]["out_c"].T
        h_out[c * BC:(c + 1) * BC] = res.results[c]["out_h"].T
    return c_out, h_out


# revision 18
# speedup vs baseline: 1.1628x; 1.0881x over previous
"""BinaryTreeLSTM Trainium2 kernel — data-parallel over the batch (tree) axis.

Layout strategy: keep every activation in [feature, row] layout on-chip.
The TensorE matmul produces PSUM [out_feat, row], which is exactly the
moving-operand layout ([K, N], K on partitions) needed by the next tree
level — so no transposes are ever needed on-device.  The host transposes
embs once (numpy) and packs/casts the gate weights.

Per core (64 trees): leaf GEMM (K=300 -> 512 feats = [c|o]), then 8
levelized compose GEMMs (K=512 = [lh|rh] feats -> 1024 = 4 gates x 256).
lh/rh are stride-2 (even/odd node) column slices of the previous level's
h buffer.  Trees are processed in chunks of 8 so leaf/L1/L2 intermediates
fit in SBUF; L3 outputs are parked, then levels 4..8 run across all 64
trees at once to keep matmul free dims large.

Throughput structure:
 - Wide levels process PAIRS of 512-row sub-chunks with [128,1024] PSUM
   tiles (2 banks) so each LDWEIGHTS feeds two matmuls — the 2-deep PE
   weight buffer otherwise serializes LDW behind the prior matmul drain.
 - Emission is software-pipelined across chunks (leaf(ch) | L2(ch-1) |
   L1(ch) | L3(ch-2)) so the PE never waits on the serial ACT->DVE chain
   between adjacent tree levels; tanh(c)->h is deferred and batched per
   level.
 - A burst of dummy matmuls at kernel start warms the PE HAM clock gate
   during the initial DMA wait.

dtypes: matmul operands bf16 (fp32 PSUM accumulate); the c cell state and
gate activations stay fp32 end-to-end.
"""

import sys

if "/opt/trn_rl_repo" not in sys.path:
    sys.path.insert(0, "/opt/trn_rl_repo")

from contextlib import ExitStack

import ml_dtypes
import numpy as np

N_CORES = 8
B, L, IN, M = 512, 256, 300, 256
BC = B // N_CORES            # trees per core
LEAF_ROWS = BC * L           # 16384 leaf rows per core
T_CHUNK = 8                  # trees per chunk
N_CHUNKS = BC // T_CHUNK
CHUNK_LEAF = T_CHUNK * L     # 2048 leaf rows per chunk

_CACHE = {}
LAST_RESULTS = None


def _build():
    import concourse.bass as bass  # noqa: F401
    import concourse.tile as tile
    from concourse import bacc, mybir

    F32 = mybir.dt.float32
    BF16 = mybir.dt.bfloat16
    SIG = mybir.ActivationFunctionType.Sigmoid
    TANH = mybir.ActivationFunctionType.Tanh
    ADD = mybir.AluOpType.add

    nc = bacc.Bacc("TRN2", target_bir_lowering=False, debug=False,
                   num_devices=N_CORES)
    emb_d = nc.dram_tensor("embs_t", [IN, LEAF_ROWS], BF16,
                           kind="ExternalInput").ap()
    wl_d = nc.dram_tensor("w_leaf", [IN, 2 * M], BF16,
                          kind="ExternalInput").ap()
    wc_d = nc.dram_tensor("w_comp", [2 * M, 4 * M], BF16,
                          kind="ExternalInput").ap()
    wl2_d = nc.dram_tensor("w_leaf2", [128, 2 * M], BF16,
                           kind="ExternalInput").ap()
    b_d = nc.dram_tensor("biases", [128, 12], F32, kind="ExternalInput").ap()
    outc_d = nc.dram_tensor("out_c", [M, BC], F32, kind="ExternalOutput").ap()
    outh_d = nc.dram_tensor("out_h", [M, BC], F32, kind="ExternalOutput").ap()

    with tile.TileContext(nc) as tc, ExitStack() as ctx:
        wpool = ctx.enter_context(tc.tile_pool(name="w", bufs=1))
        xpool = ctx.enter_context(tc.tile_pool(name="x", bufs=4))
        lvl = ctx.enter_context(tc.tile_pool(name="lvl", bufs=1))
        gp = ctx.enter_context(tc.tile_pool(name="g", bufs=2))
        pp = ctx.enter_context(tc.tile_pool(name="ps", bufs=1, space="PSUM"))

        # --- weights / biases (resident); issued on the ACT HWDGE queue so
        # the SP queue leads with the first leaf's embs tiles ---
        wl = [wpool.tile([128, 2 * M], BF16, name=f"wl{k}", tag=f"wl{k}")
              for k in range(2)]
        nc.scalar.dma_start(wl[0][:], wl_d[0:128, :])
        nc.scalar.dma_start(wl[1][:], wl_d[128:256, :])
        wl2 = wpool.tile([128, 2 * M], BF16, name="wl2", tag="wl2")
        nc.scalar.dma_start(wl2[:], wl2_d[:, :])
        wc = [wpool.tile([128, 4 * M], BF16, name=f"wc{k}", tag=f"wc{k}")
              for k in range(4)]
        bias = wpool.tile([128, 12], F32, name="bias", tag="bias")
        nc.scalar.dma_start(bias[:], b_d[:, :])

        # PSUM: eight [128, 512] tiles = one bank each
        def ps_tile(i, n=512):
            t = pp.tile([128, 512], F32, name=f"ps{i}", tag=f"ps{i}")
            return t[:, :n] if n < 512 else t

        # --- HAM warm-up: dummy matmuls during the startup DMA wait ---
        dummy = wpool.tile([128, 512], BF16, name="dummy", tag="dummy")
        nc.gpsimd.memset(dummy[:], 0.0)
        for i in range(24):
            nc.tensor.matmul(ps_tile(i % 8), dummy[:, 0:128],
                             dummy[:], start=True, stop=True)

        # --- parked L3 state (all trees) ---
        pk_w = BC * 32
        pk_h = [lvl.tile([128, pk_w], BF16, name=f"pkh{p}", tag=f"pkh{p}")
                for p in range(2)]
        pk_c = [lvl.tile([128, pk_w], F32, name=f"pkc{p}", tag=f"pkc{p}")
                for p in range(2)]

        # gate m-tile order: the gates needed first by the DVE c-chain
        # (i, u, lf, rf of part-half 0) drain first.
        MT_ORDER = (0, 6, 2, 4, 1, 7, 3, 5)

        def _gates_and_chain(ps, ce, houts, couts, n, defer_h):
            """ps[mt]: psum APs [128, n]; ce[p]: even/odd c view
            [128, 2, n]; gates + c-chain for one <=512-row sub-chunk."""
            g = [None] * 8
            for mt in MT_ORDER:
                gt = gp.tile([128, 512], F32, name=f"g{mt}", tag=f"g{mt}")
                fn = TANH if mt >= 6 else SIG
                nc.scalar.activation(gt[:, :n], ps[mt], fn,
                                     bias=bias[:, 4 + mt:5 + mt])
                g[mt] = gt
            for p in range(2):
                t1 = gp.tile([128, 512], F32, name=f"t1_{p}", tag=f"t1_{p}",
                             bufs=1)
                nc.vector.tensor_mul(t1[:, :n], g[0 + p][:, :n],
                                     g[6 + p][:, :n])
                t2 = gp.tile([128, 512], F32, name=f"t2_{p}", tag=f"t2_{p}",
                             bufs=1)
                nc.vector.tensor_mul(t2[:, :n], g[2 + p][:, :n],
                                     ce[p][:, 0, :])
                t3 = gp.tile([128, 512], F32, name=f"t3_{p}", tag=f"t3_{p}",
                             bufs=1)
                nc.vector.tensor_mul(t3[:, :n], g[4 + p][:, :n],
                                     ce[p][:, 1, :])
                s12 = gp.tile([128, 512], F32, name=f"s12_{p}",
                              tag=f"s12_{p}", bufs=1)
                nc.vector.tensor_add(s12[:, :n], t1[:, :n], t2[:, :n])
                nc.vector.tensor_add(couts[p], s12[:, :n], t3[:, :n])
                if not defer_h:
                    nc.scalar.activation(houts[p], couts[p], TANH)

        def compose(hp, cp, houts, couts, n, defer_h=False):
            """One compose sub-chunk (n <= 512 output rows)."""
            he = [hp[p].rearrange("q (n two) -> q two n", two=2)
                  for p in range(2)]
            ce = [cp[p].rearrange("q (n two) -> q two n", two=2)
                  for p in range(2)]
            rhs = [he[0][:, 0, :], he[1][:, 0, :],
                   he[0][:, 1, :], he[1][:, 1, :]]
            ps = [ps_tile(mt, n) for mt in range(8)]
            for mt in MT_ORDER:
                for k in range(4):
                    nc.tensor.matmul(ps[mt],
                                     wc[k][:, mt * 128:(mt + 1) * 128],
                                     rhs[k], start=(k == 0), stop=(k == 3))
            _gates_and_chain(ps, ce, houts, couts, n, defer_h)

        st = {}  # (level, ch) -> (h_tiles, c_tiles)

        def emit_leaf(ch):
            h_lf = [lvl.tile([128, CHUNK_LEAF], BF16, name=f"hlf{p}",
                             tag=f"hlf{p}") for p in range(2)]
            c_lf = [lvl.tile([128, CHUNK_LEAF], F32, name=f"clf{p}",
                             tag=f"clf{p}") for p in range(2)]
            for s in range(CHUNK_LEAF // 512):
                col0 = ch * CHUNK_LEAF + s * 512
                xk = [xpool.tile([128, 512], BF16, name=f"xk{k}",
                                 tag=f"xk{k}") for k in range(3)]
                nc.sync.dma_start(xk[0][:], emb_d[0:128, col0:col0 + 512])
                nc.sync.dma_start(xk[1][:], emb_d[128:256, col0:col0 + 512])
                nc.sync.dma_start(xk[2][:44, :],
                                  emb_d[256:300, col0:col0 + 512])
                nc.sync.dma_start(xk[2][64:108, :],
                                  emb_d[256:300, col0:col0 + 512])
                pso = 4 * (s % 2)
                ps = [ps_tile(pso + mt) for mt in range(4)]
                for mt in range(4):
                    for k in range(2):
                        nc.tensor.matmul(
                            ps[mt],
                            wl[k][:, mt * 128:(mt + 1) * 128],
                            xk[k][:], start=(k == 0), stop=False)
                # K=44 tail: two m-tiles run concurrently in disjoint PE
                # row groups (rows 0-43 and 64-107)
                for mt in range(0, 4, 2):
                    nc.tensor.matmul(
                        ps[mt], wl2[0:44, mt * 128:(mt + 1) * 128],
                        xk[2][0:44, :], start=False, stop=True,
                        tile_position=(0, 0))
                    nc.tensor.matmul(
                        ps[mt + 1],
                        wl2[64:108, (mt + 1) * 128:(mt + 2) * 128],
                        xk[2][64:108, :], start=False, stop=True,
                        tile_position=(64, 0))
                dst = slice(s * 512, (s + 1) * 512)
                for p in range(2):
                    tcell = gp.tile([128, 512], F32, name=f"lf_tc{p}",
                                    tag=f"lf_tc{p}")
                    nc.scalar.activation(tcell[:], ps[p], TANH,
                                         bias=bias[:, p:p + 1])
                    to = gp.tile([128, 512], F32, name=f"lf_to{p}",
                                 tag=f"lf_to{p}")
                    nc.scalar.activation(to[:], ps[2 + p], SIG,
                                         bias=bias[:, 2 + p:3 + p])
                    nc.vector.tensor_scalar(c_lf[p][:, dst], ps[p],
                                            bias[:, p:p + 1], None, ADD)
                    nc.vector.tensor_mul(h_lf[p][:, dst], to[:], tcell[:])
            st[(0, ch)] = (h_lf, c_lf)

        def emit_level(li, ch):
            """Compose level li (1..3) of chunk ch."""
            prev_h, prev_c = st.pop((li - 1, ch))
            rows = CHUNK_LEAF >> li
            if li < 3:
                nh = [lvl.tile([128, rows], BF16, name=f"h{li}_{p}",
                               tag=f"h{li}_{p}", bufs=2) for p in range(2)]
                ncr = [lvl.tile([128, rows], F32, name=f"c{li}_{p}",
                                tag=f"c{li}_{p}", bufs=2) for p in range(2)]
                off = 0
            else:
                nh, ncr, off = pk_h, pk_c, ch * (CHUNK_LEAF // 8)
            for j in range(max(1, rows // 512)):
                n = min(512, rows - j * 512)
                hp = [prev_h[p][:, j * 1024:j * 1024 + 2 * n]
                      for p in range(2)]
                cp = [prev_c[p][:, j * 1024:j * 1024 + 2 * n]
                      for p in range(2)]
                houts = [nh[p][:, off + j * 512:off + j * 512 + n]
                         for p in range(2)]
                couts = [ncr[p][:, off + j * 512:off + j * 512 + n]
                         for p in range(2)]
                compose(hp, cp, houts, couts, n, defer_h=True)
            for p in range(2):
                nc.scalar.activation(nh[p][:, off:off + rows],
                                     ncr[p][:, off:off + rows], TANH)
            if li < 3:
                st[(li, ch)] = (nh, ncr)

        # software-pipelined chunk schedule
        for ch in range(N_CHUNKS):
            emit_leaf(ch)
            if ch == 0:
                for k in range(4):
                    nc.scalar.dma_start(wc[k][:],
                                        wc_d[k * 128:(k + 1) * 128, :])
            if ch >= 1:
                emit_level(2, ch - 1)
            emit_level(1, ch)
            if ch >= 2:
                emit_level(3, ch - 2)
        emit_level(2, N_CHUNKS - 1)
        emit_level(3, N_CHUNKS - 2)

        # ---- tail levels 4..8 across all trees ----
        def tail_tiles(li, rows):
            nh = [lvl.tile([128, rows], BF16, name=f"h{li}_{p}",
                           tag=f"h{li}_{p}") for p in range(2)]
            ncr = [lvl.tile([128, rows], F32, name=f"c{li}_{p}",
                            tag=f"c{li}_{p}") for p in range(2)]
            return nh, ncr

        def tail_sub(prev, cur, j, n):
            prev_h, prev_c = prev
            nh, ncr = cur
            hp = [prev_h[p][:, j * 2 * n:j * 2 * n + 2 * n]
                  for p in range(2)]
            cp = [prev_c[p][:, j * 2 * n:j * 2 * n + 2 * n]
                  for p in range(2)]
            houts = [nh[p][:, j * n:j * n + n] for p in range(2)]
            couts = [ncr[p][:, j * n:j * n + n] for p in range(2)]
            compose(hp, cp, houts, couts, n, defer_h=True)
            for p in range(2):
                nc.scalar.activation(houts[p], couts[p], TANH)

        l4 = tail_tiles(4, BC * 16)
        l5 = tail_tiles(5, BC * 8)
        l6 = tail_tiles(6, BC * 4)
        l7 = tail_tiles(7, BC * 2)
        tail_sub((pk_h, pk_c), l4, 0, 512)
        emit_level(3, N_CHUNKS - 1)
        tail_sub((pk_h, pk_c), l4, 1, 512)
        tail_sub(l4, l5, 0, 256)
        tail_sub(l4, l5, 1, 256)
        tail_sub(l5, l6, 0, 128)
        tail_sub(l5, l6, 1, 128)
        tail_sub(l6, l7, 0, 64)
        tail_sub(l6, l7, 1, 64)
        stage_c = [lvl.tile([128, BC], F32, name=f"stc{p}", tag=f"stc{p}")
                   for p in range(2)]
        stage_h = [lvl.tile([128, BC], F32, name=f"sth{p}", tag=f"sth{p}")
                   for p in range(2)]
        for j in range(2):
            compose([l7[0][p][:, j * 64:j * 64 + 64] for p in range(2)],
                    [l7[1][p][:, j * 64:j * 64 + 64] for p in range(2)],
                    [stage_h[p][:, j * 32:(j + 1) * 32] for p in range(2)],
                    [stage_c[p][:, j * 32:(j + 1) * 32] for p in range(2)],
                    32)
        for p in range(2):
            nc.sync.dma_start(outc_d[p * 128:(p + 1) * 128, :], stage_c[p][:])
            nc.sync.dma_start(outh_d[p * 128:(p + 1) * 128, :], stage_h[p][:])

    nc.compile()
    return nc


def _prep_inputs(embs, cx_w, cx_b, ox_w, ox_b, lh_w, lh_b, rh_w, rh_b):
    bf16 = ml_dtypes.bfloat16
    w_leaf = np.ascontiguousarray(
        np.concatenate([cx_w, ox_w], axis=0).T).astype(bf16)      # [300, 512]
    w_leaf2 = np.zeros((128, 2 * M), bf16)
    w_leaf2[0:44] = w_leaf[256:300]
    w_leaf2[64:108] = w_leaf[256:300]
    w_comp = np.ascontiguousarray(np.concatenate(
        [lh_w.reshape(4 * M, M).T, rh_w.reshape(4 * M, M).T],
        axis=0)).astype(bf16)                                     # [512, 1024]
    b_all = np.concatenate(
        [cx_b, ox_b, (lh_b + rh_b).reshape(-1)]).astype(np.float32)
    biases = np.ascontiguousarray(b_all.reshape(12, 128).T)       # [128, 12]
    embs_sh = np.asarray(embs, np.float32).reshape(N_CORES, LEAF_ROWS, IN)
    in_maps = []
    for c in range(N_CORES):
        xt = embs_sh[c].T.astype(bf16)                            # [300, 16384]
        in_maps.append({"embs_t": np.ascontiguousarray(xt),
                        "w_leaf": w_leaf, "w_leaf2": w_leaf2,
                        "w_comp": w_comp, "biases": biases})
    return in_maps


def kernel(embs, cx_w, cx_b, ox_w, ox_b, lh_w, lh_b, rh_w, rh_b):
    global LAST_RESULTS
    from concourse.bass_utils import run_bass_kernel_spmd

    if "nc" not in _CACHE:
        _CACHE["nc"] = _build()
    nc = _CACHE["nc"]
    in_maps = _prep_inputs(embs, cx_w, cx_b, ox_w, ox_b,
                           lh_w, lh_b, rh_w, rh_b)
    res = run_bass_kernel_spmd(nc, in_maps, core_ids=list(range(N_CORES)))
    LAST_RESULTS = res
    c_out = np.empty((B, M), np.float32)
    h_out = np.empty((B, M), np.float32)
    for c in range(N_CORES):
        c_out[c * BC:(c + 1) * BC] = res.results[c]["out_c"].T
        h_out[c * BC:(c + 1) * BC] = res.results[c]["out_h"].T
    return c_out, h_out
